# revision 19
# baseline (speedup 1.0000x reference)
"""Trainium2 Bass kernel for nn_DeepSetLayer (GNN attention message passing).

Design (8 NeuronCores, graph-parallel by destination node):
  Host: append self-loops; bin-pack dst nodes into 8 cores x 50 blocks
  (<=128 dsts, <=13*128 edges per block); lay out three edge-ordered
  views of the node features (pure layout, no float math):
    XEM [edge, feat]  - src features, edge-major  (aggregation matmul lhsT)
    XTG [feat, edge]  - src features, feat-major  (per-edge q matmul lhsT)
    XDT [feat, edge]  - dst features, feat-major  (per-edge k matmul lhsT)
  Device (one SPMD program, no collectives), per block:
    psq/psk = bias (ones-row matmul) + 13 accum matmuls    (PE, all bf16)
    q = tanh(psq), k = copy(psk)                           (ACT from PSUM)
    scores = reduce(q*k), expb = exp(scores/sqrt(S))       (DVE 2x + ACT)
    per tile: sw[e,d] = (iota==dstloc)*expb  (DVE 4x, all-bf16 operands)
      att_T += XEM_t.T @ sw ; seg += sw.T @ ones           (PE, PSUM accum)
    epilogue: pa = attT@W2T; pb = x@W1T + b2 (bf16 matmuls)
      xpre = pa*(1/seg) + pb  (one DVE STT), ssq += xpre^2 (ACT Square accum)
  Final phase: one Sqrt/reciprocal over all 50 blocks' ssq (single ACT
  table load), then relu(xpre*rinv) per block (DVE 4x) -> bf16 out DMA.
  Host: inverse-permute per-core outputs into the full [N, F] result.
"""

import math
import sys

sys.path.insert(0, "/opt/trn_rl_repo")

import heapq

import ml_dtypes
import numpy as np

import concourse.bacc as bacc
import concourse.bass as bass
import concourse.mybir as mybir
import concourse.tile as tile
from concourse.bass_utils import run_bass_kernel_spmd

N = 50000
E = 600000
F = 128
S = 12
NCORES = 8
B = 50            # blocks per core
TB = 13           # 128-edge tiles per block
DSTS = B * 128    # 6400 padded dst slots per core
G = 5             # blocks per DMA group
NG = B // G
NPL = B * TB      # total edge tiles (planes) per core
GPL = G * TB      # planes per group
INV_SQRT_S = 1.0 / math.sqrt(float(S))

f32 = mybir.dt.float32
bf16 = mybir.dt.bfloat16
bf16_np = ml_dtypes.bfloat16

_compiled = {}


def _pack_bins(deg):
    """Assign each dst to one of NCORES*B bins (<=128 dsts, <=TB*128 edges),
    balancing edge counts."""
    nbins = NCORES * B
    order = np.argsort(-deg, kind="stable")
    b_e = np.zeros(nbins, np.int64)
    b_n = np.zeros(nbins, np.int64)
    bins_dsts = [[] for _ in range(nbins)]
    heap = [(0, b) for b in range(nbins)]
    heapq.heapify(heap)
    for dst in order:
        dst = int(dst)
        d = int(deg[dst])
        stash = []
        while True:
            ec, b = heapq.heappop(heap)
            if ec != b_e[b]:
                continue
            if b_n[b] < 128 and b_e[b] + d <= TB * 128:
                break
            stash.append((ec, b))
        bins_dsts[b].append(dst)
        b_e[b] += d
        b_n[b] += 1
        if b_n[b] < 128:
            heapq.heappush(heap, (int(b_e[b]), b))
        for it in stash:
            heapq.heappush(heap, it)
    return bins_dsts


def _host_prep(node_data, src, dst):
    x = np.ascontiguousarray(np.asarray(node_data, np.float32))
    loops = np.arange(N, dtype=np.int64)
    s_all = np.concatenate([np.asarray(src, np.int64), loops])
    d_all = np.concatenate([np.asarray(dst, np.int64), loops])

    deg = np.bincount(d_all, minlength=N)
    bins_dsts = _pack_bins(deg)

    perm = np.full(NCORES * DSTS, -1, dtype=np.int64)
    for b, dlist in enumerate(bins_dsts):
        core, blk = divmod(b, B)
        base = core * DSTS + blk * 128
        perm[base : base + len(dlist)] = dlist

    # CSR of edges by dst
    eorder = np.argsort(d_all, kind="stable")
    indptr = np.zeros(N + 1, dtype=np.int64)
    np.cumsum(deg, out=indptr[1:])
    s_sorted = s_all[eorder]

    node_bf = np.ascontiguousarray(x.astype(bf16_np))

    per_core = []
    for core in range(NCORES):
        srcmat = np.zeros((128, NPL), np.int64)
        dstmat = np.zeros((128, NPL), np.int64)
        dstloc = np.full((128, NPL), 992.0, dtype=np.float32)

        for blk in range(B):
            dlist = bins_dsts[core * B + blk]
            ss, dd_, dl = [], [], []
            for j, d0 in enumerate(dlist):
                es = s_sorted[indptr[d0] : indptr[d0 + 1]]
                ss.append(es)
                dd_.append(np.full(len(es), d0, np.int64))
                dl.append(np.full(len(es), j, np.float32))
            ss = np.concatenate(ss) if ss else np.zeros(0, np.int64)
            dd_ = np.concatenate(dd_) if dd_ else np.zeros(0, np.int64)
            dl = np.concatenate(dl) if dl else np.zeros(0, np.float32)
            ne = len(ss)
            assert ne <= TB * 128, f"block overflow {ne}"
            sp = np.zeros(TB * 128, np.int64)
            sp[:ne] = ss
            dp = np.zeros(TB * 128, np.int64)
            dp[:ne] = dd_
            lp = np.full(TB * 128, 992.0, np.float32)
            lp[:ne] = dl
            # edge slot j -> (plane j//128, partition j%128)
            pl0 = blk * TB
            srcmat[:, pl0 : pl0 + TB] = sp.reshape(TB, 128).T
            dstmat[:, pl0 : pl0 + TB] = dp.reshape(TB, 128).T
            dstloc[:, pl0 : pl0 + TB] = lp.reshape(TB, 128).T

        # edge-ordered feature streams
        g_src = node_bf[srcmat]                 # [128 e, NPL, F]
        g_dst = node_bf[dstmat]                 # [128 e, NPL, F]
        xem = np.ascontiguousarray(g_src.reshape(128, NPL * F))
        xtg = np.ascontiguousarray(
            g_src.transpose(2, 1, 0).reshape(128, NPL * 128)
        )  # [feat, (plane, edge)]
        xdt = np.ascontiguousarray(
            g_dst.transpose(2, 1, 0).reshape(128, NPL * 128)
        )

        nshT = np.zeros((F, DSTS), np.float32)
        sl = perm[core * DSTS : (core + 1) * DSTS]
        valid = sl >= 0
        nshT[:, valid] = x[sl[valid]].T

        per_core.append(
            dict(
                xem=xem,
                xtg=xtg,
                xdt=xdt,
                nshT=nshT.astype(bf16_np),
                dstloc=dstloc,
            )
        )

    return per_core, perm


def _build_nc():
    nc = bacc.Bacc(
        "TRN2",
        target_bir_lowering=False,
        debug=False,
        enable_asserts=False,
        num_devices=NCORES,
    )
    AF = mybir.ActivationFunctionType
    OP = mybir.AluOpType

    xem_d = nc.dram_tensor("xem", [128, NPL * F], bf16, kind="ExternalInput")
    xtg_d = nc.dram_tensor("xtg", [128, NPL * F], bf16, kind="ExternalInput")
    xdt_d = nc.dram_tensor("xdt", [128, NPL * F], bf16, kind="ExternalInput")
    nshT_d = nc.dram_tensor("nshT", [F, DSTS], bf16, kind="ExternalInput")
    dstloc_d = nc.dram_tensor("dstloc", [128, NPL], f32, kind="ExternalInput")
    wqT_d = nc.dram_tensor("wqT", [F, S], bf16, kind="ExternalInput")
    wkT_d = nc.dram_tensor("wkT", [F, S], bf16, kind="ExternalInput")
    w1T_d = nc.dram_tensor("w1T", [F, F], bf16, kind="ExternalInput")
    w2T_d = nc.dram_tensor("w2T", [F, F], bf16, kind="ExternalInput")
    bqk_d = nc.dram_tensor("bqk", [1, TB * 2 * S], bf16, kind="ExternalInput")
    b2r_d = nc.dram_tensor("b2r", [1, F], bf16, kind="ExternalInput")
    iota_d = nc.dram_tensor("iota", [128, 128], bf16, kind="ExternalInput")
    out_d = nc.dram_tensor("out", [DSTS, F], bf16, kind="ExternalOutput")

    with tile.TileContext(nc) as tc:
        with tc.tile_pool(name="const", bufs=1) as const:
            nshT = const.tile([F, DSTS], bf16)
            nc.sync.dma_start(nshT[:], nshT_d[:])
            dstloc = const.tile([128, NPL], f32)
            nc.sync.dma_start(dstloc[:], dstloc_d[:])
            wqT = const.tile([F, S], bf16)
            nc.sync.dma_start(wqT[:], wqT_d[:])
            wkT = const.tile([F, S], bf16)
            nc.sync.dma_start(wkT[:], wkT_d[:])
            w1T = const.tile([F, F], bf16)
            nc.sync.dma_start(w1T[:], w1T_d[:])
            w2T = const.tile([F, F], bf16)
            nc.sync.dma_start(w2T[:], w2T_d[:])
            bqk = const.tile([1, TB * 2 * S], bf16)
            nc.sync.dma_start(bqk[:], bqk_d[:])
            b2r = const.tile([1, F], bf16)
            nc.sync.dma_start(b2r[:], b2r_d[:])
            iota = const.tile([128, 128], bf16)
            nc.sync.dma_start(iota[:], iota_d[:])
            ones_col = const.tile([128, 1], bf16)
            nc.vector.memset(ones_col[:], 1.0)
            ones_row = const.tile([1, 128], bf16)
            nc.vector.memset(ones_row[:], 1.0)
            # per-core accumulators for the deferred l2-norm phase
            xpre_bf = const.tile([128, B * F], bf16)
            ssq_all = const.tile([128, B], f32)
            pb_all = const.tile([128, B * F], f32)
            ot_all = const.tile([128, B * F], bf16)

            with (
                tc.tile_pool(name="xe", bufs=2) as xep,
                tc.tile_pool(name="xt", bufs=2) as xtp,
                tc.tile_pool(name="xd", bufs=2) as xdp,
                tc.tile_pool(name="wk3", bufs=6) as wk3,
                tc.tile_pool(name="swp", bufs=8) as swp,
            ):
                streams = {}

                def issue_group(g):
                    csl = slice(g * GPL * F, (g + 1) * GPL * F)
                    XEM = xep.tile([128, GPL * F], bf16, tag="XEM")
                    nc.sync.dma_start(XEM[:], xem_d[:, csl])
                    XTG = xtp.tile([128, GPL * F], bf16, tag="XTG")
                    nc.sync.dma_start(XTG[:], xtg_d[:, csl])
                    XDT = xdp.tile([128, GPL * F], bf16, tag="XDT")
                    nc.sync.dma_start(XDT[:], xdt_d[:, csl])
                    streams[g] = (XEM, XTG, XDT)

                def emit_qk(b):
                    """q/k matmuls + tanh/copy for block b (lookahead)."""
                    g, bb = divmod(b, G)
                    _, XTG, XDT = streams[g]
                    psqk = ps_qk.tile([128, TB, 2 * S], f32, tag="psqk")
                    nc.tensor.matmul(
                        psqk[:].rearrange("p t s -> p (t s)"),
                        ones_row[:], bqk[:],
                        start=True, stop=False, skip_group_check=True,
                    )
                    for t in range(TB):
                        fsl = slice((bb * TB + t) * F, (bb * TB + t + 1) * F)
                        nc.tensor.matmul(
                            psqk[:, t, 0:S], XTG[:, fsl], wqT[:],
                            start=False, stop=True, skip_group_check=True,
                        )
                        nc.tensor.matmul(
                            psqk[:, t, S : 2 * S], XDT[:, fsl], wkT[:],
                            start=False, stop=True, skip_group_check=True,
                        )
                    q_sb = wk3.tile([128, TB, S], bf16, tag="qsb")
                    nc.scalar.activation(q_sb[:], psqk[:, :, 0:S], AF.Tanh)
                    k_sb = wk3.tile([128, TB, S], bf16, tag="ksb")
                    nc.scalar.activation(
                        k_sb[:], psqk[:, :, S : 2 * S], AF.Copy
                    )
                    return q_sb, k_sb

                def emit_rest(b, qk):
                    g, bb = divmod(b, G)
                    XEM, _, _ = streams[g]
                    q_sb, k_sb = qk
                    prod = wk3.tile([128, TB, S], bf16, tag="prod")
                    nc.vector.tensor_tensor(prod[:], q_sb[:], k_sb[:], OP.mult)
                    scores = wk3.tile([128, TB], f32, tag="scores")
                    nc.vector.tensor_reduce(
                        scores[:], prod[:], mybir.AxisListType.X, OP.add
                    )
                    expb = wk3.tile([128, TB], f32, tag="expb")
                    nc.scalar.activation(
                        expb[:], scores[:], AF.Exp, scale=INV_SQRT_S
                    )

                    att = ps_att.tile([F, 128], f32, tag="att")
                    paseg = ps_ab.tile([128, F + 1], f32, tag="paseg")
                    seg = paseg[:, F : F + 1]
                    pa = paseg[:, 0:F]
                    for t in range(TB):
                        fsl = slice((bb * TB + t) * F, (bb * TB + t + 1) * F)
                        sw = swp.tile([128, 128], bf16, tag="sw")
                        nc.vector.tensor_scalar(
                            sw[:],
                            iota[:],
                            dstloc[:, b * TB + t : b * TB + t + 1],
                            expb[:, t : t + 1],
                            OP.is_equal,
                            OP.mult,
                        )
                        nc.tensor.matmul(
                            att[:], XEM[:, fsl], sw[:],
                            start=(t == 0), stop=(t == TB - 1),
                        )
                        nc.tensor.matmul(
                            seg, sw[:], ones_col[:],
                            start=(t == 0), stop=(t == TB - 1),
                            skip_group_check=True,
                        )

                    attT_sb = wk3.tile([F, 128], bf16, tag="attTsb")
                    nc.scalar.activation(attT_sb[:], att[:], AF.Copy)
                    nc.tensor.matmul(
                        pa, attT_sb[:], w2T[:], start=True, stop=True,
                        skip_group_check=True,
                    )
                    rec = wk3.tile([128, 1], f32, tag="rec")
                    nc.vector.reciprocal(rec[:], seg)
                    # xpre = pa/seg + pb, stored bf16 for the final phase
                    nc.vector.scalar_tensor_tensor(
                        xpre_bf[:, b * F : (b + 1) * F],
                        pa, rec[:], pb_all[:, b * F : (b + 1) * F],
                        OP.mult, OP.add,
                    )
                    sq_scr = wk3.tile([128, F], bf16, tag="sqscr")
                    nc.scalar.activation(
                        sq_scr[:],
                        xpre_bf[:, b * F : (b + 1) * F],
                        AF.Square,
                        accum_out=ssq_all[:, b : b + 1],
                    )

                out_v = out_d[:].rearrange("(b p) f -> p b f", p=128)
                ot_v = ot_all[:].rearrange("p (b f) -> p b f", f=F)

                def emit_norm(b0, b1):
                    """Normalize + relu + out-DMA for blocks [b0, b1)."""
                    nrm = wk3.tile([128, B], f32, tag="nrm")
                    nc.scalar.activation(
                        nrm[:, b0:b1], ssq_all[:, b0:b1], AF.Sqrt
                    )
                    rin = wk3.tile([128, B], f32, tag="rin")
                    nc.vector.reciprocal(rin[:, b0:b1], nrm[:, b0:b1])
                    for b in range(b0, b1):
                        nc.vector.tensor_scalar(
                            ot_all[:, b * F : (b + 1) * F],
                            xpre_bf[:, b * F : (b + 1) * F],
                            rin[:, b : b + 1],
                            0.0,
                            OP.mult,
                            OP.max,
                        )
                    CB = 10
                    for c in range(b0 // CB, b1 // CB):
                        nc.sync.dma_start(
                            out_v[:, c * CB : (c + 1) * CB, :],
                            ot_v[:, c * CB : (c + 1) * CB, :],
                        )

                issue_group(0)
                # upfront pb = x@W1T + b2 (PE busy while streams land)
                with tc.tile_pool(name="ps_pb", bufs=2, space="PSUM") as ps_pb:
                    for b0 in range(B):
                        pbp = ps_pb.tile([128, F], f32, tag="pbp")
                        nc.tensor.matmul(
                            pbp[:],
                            nshT[:, b0 * 128 : (b0 + 1) * 128],
                            w1T[:],
                            start=True,
                            stop=False,
                        )
                        nc.tensor.matmul(
                            pbp[:], ones_row[:, :F], b2r[:],
                            start=False, stop=True,
                        )
                        nc.scalar.activation(
                            pb_all[:, b0 * F : (b0 + 1) * F], pbp[:],
                            AF.Copy,
                        )

                # software-pipelined emission: q/k of block b+1 queued on
                # PE before block b's sw-gated att/seg, so neither engine
                # ping-pongs on the other once per block.
                with (
                    tc.tile_pool(name="ps_qk", bufs=3, space="PSUM") as ps_qk,
                    tc.tile_pool(name="ps_att", bufs=2, space="PSUM") as ps_att,
                    tc.tile_pool(name="ps_ab", bufs=3, space="PSUM") as ps_ab,
                ):
                    qk = emit_qk(0)
                    for b in range(B):
                        g = b // G
                        if b % G == 0 and g + 1 < NG:
                            issue_group(g + 1)
                        qk_next = emit_qk(b + 1) if b + 1 < B else None
                        emit_rest(b, qk)
                        qk = qk_next
                    emit_norm(0, B)


    nc.compile()
    return nc


def get_nc():
    if "nc" not in _compiled:
        _compiled["nc"] = _build_nc()
    return _compiled["nc"]


def _make_in_maps(node_data, src, dst, Wq, bq, Wk, bk, W1, W2, b2):
    per_core, perm = _host_prep(node_data, src, dst)
    consts = dict(
        wqT=np.ascontiguousarray(np.asarray(Wq, np.float32).T).astype(bf16_np),
        wkT=np.ascontiguousarray(np.asarray(Wk, np.float32).T).astype(bf16_np),
        w1T=np.ascontiguousarray(np.asarray(W1, np.float32).T).astype(bf16_np),
        w2T=np.ascontiguousarray(np.asarray(W2, np.float32).T).astype(bf16_np),
        bqk=np.tile(
            np.concatenate([np.asarray(bq, np.float32),
                            np.asarray(bk, np.float32)])[None, :],
            (1, TB),
        ).astype(bf16_np),
        b2r=np.ascontiguousarray(
            np.asarray(b2, np.float32)[None, :]
        ).astype(bf16_np),
        iota=np.tile(
            np.arange(128, dtype=np.float32)[None, :], (128, 1)
        ).astype(bf16_np),
    )
    in_maps = []
    for core in range(NCORES):
        m = dict(consts)
        m.update(per_core[core])
        in_maps.append(m)
    return in_maps, perm


def run(node_data, src, dst, Wq, bq, Wk, bk, W1, W2, b2, trace=False,
        tmpdir=None, n_runs=1):
    in_maps, perm = _make_in_maps(
        node_data, src, dst, Wq, bq, Wk, bk, W1, W2, b2
    )
    nc = get_nc()
    res = None
    for r in range(n_runs):
        td = tmpdir if (tmpdir is None or n_runs == 1) else f"{tmpdir}_{r}"
        if td is not None:
            import os
            import shutil
            shutil.rmtree(td, ignore_errors=True)
            os.makedirs(td, exist_ok=True)
        rr = run_bass_kernel_spmd(
            nc, in_maps, list(range(NCORES)), trace=trace, tmpdir=td
        )
        if rr.exec_time_ns is not None:
            print(f"  run {r}: {rr.exec_time_ns} ns")
        if res is None or (
            rr.exec_time_ns is not None
            and res.exec_time_ns is not None
            and rr.exec_time_ns < res.exec_time_ns
        ):
            res = rr
    out = np.zeros((N, F), dtype=np.float32)
    for core in range(NCORES):
        o = np.asarray(res.results[core]["out"], dtype=np.float32)
        sl = perm[core * DSTS : (core + 1) * DSTS]
        valid = sl >= 0
        out[sl[valid]] = o[valid]
    return out, res


def kernel(node_data, src, dst, Wq, bq, Wk, bk, W1, W2, b2):
    out, _ = run(node_data, src, dst, Wq, bq, Wk, bk, W1, W2, b2, trace=False)
    return out


if __name__ == "__main__":
    nc = get_nc()
    print("compiled OK")


# revision 20
# speedup vs baseline: 1.2238x; 1.2238x over previous
"""Trainium2 Bass kernel for nn_DeepSetLayer (GNN attention message passing).

Design (8 NeuronCores, graph-parallel by destination node):
  Host: append self-loops; bin-pack dst nodes into 8 cores x 50 blocks
  (<=128 dsts, <=13*128 edges per block); lay out three edge-ordered
  views of the node features (pure layout, no float math):
    XEM [edge, feat]  - src features, edge-major  (aggregation matmul lhsT)
    XTG [feat, edge]  - src features, feat-major  (per-edge q matmul lhsT)
    XDT [feat, edge]  - dst features, feat-major  (per-edge k matmul lhsT)
  Device (one SPMD program, no collectives), per block:
    psq/psk = bias (ones-row matmul) + 13 accum matmuls    (PE, all bf16)
    q = tanh(psq), k = copy(psk)                           (ACT from PSUM)
    scores = reduce(q*k), expb = exp(scores/sqrt(S))       (DVE 2x + ACT)
    per tile: sw[e,d] = (iota==dstloc)*expb  (DVE 4x, all-bf16 operands)
      att_T += XEM_t.T @ sw ; seg += sw.T @ ones           (PE, PSUM accum)
    epilogue: pa = attT@W2T; pb = x@W1T + b2 (bf16 matmuls)
      xpre = pa*(1/seg) + pb  (one DVE STT), ssq += xpre^2 (ACT Square accum)
  Final phase: one Sqrt/reciprocal over all 50 blocks' ssq (single ACT
  table load), then relu(xpre*rinv) per block (DVE 4x) -> bf16 out DMA.
  Host: inverse-permute per-core outputs into the full [N, F] result.
"""

import math
import sys

sys.path.insert(0, "/opt/trn_rl_repo")

import heapq

import ml_dtypes
import numpy as np

import concourse.bacc as bacc
import concourse.bass as bass
import concourse.mybir as mybir
import concourse.tile as tile
from concourse.bass_utils import run_bass_kernel_spmd

N = 50000
E = 600000
F = 128
S = 12
NCORES = 8
B = 50            # blocks per core
TB = 13           # 128-edge tiles per block
DSTS = B * 128    # 6400 padded dst slots per core
G = 5             # blocks per DMA group
NG = B // G
NPL = B * TB      # total edge tiles (planes) per core
GPL = G * TB      # planes per group
INV_SQRT_S = 1.0 / math.sqrt(float(S))

f32 = mybir.dt.float32
bf16 = mybir.dt.bfloat16
bf16_np = ml_dtypes.bfloat16

_compiled = {}


def _pack_bins(deg):
    """Assign each dst to one of NCORES*B bins (<=128 dsts, <=TB*128 edges),
    balancing edge counts."""
    nbins = NCORES * B
    order = np.argsort(-deg, kind="stable")
    b_e = np.zeros(nbins, np.int64)
    b_n = np.zeros(nbins, np.int64)
    bins_dsts = [[] for _ in range(nbins)]
    heap = [(0, b) for b in range(nbins)]
    heapq.heapify(heap)
    for dst in order:
        dst = int(dst)
        d = int(deg[dst])
        stash = []
        while True:
            ec, b = heapq.heappop(heap)
            if ec != b_e[b]:
                continue
            if b_n[b] < 128 and b_e[b] + d <= TB * 128:
                break
            stash.append((ec, b))
        bins_dsts[b].append(dst)
        b_e[b] += d
        b_n[b] += 1
        if b_n[b] < 128:
            heapq.heappush(heap, (int(b_e[b]), b))
        for it in stash:
            heapq.heappush(heap, it)
    return bins_dsts


def _host_prep(node_data, src, dst):
    x = np.ascontiguousarray(np.asarray(node_data, np.float32))
    loops = np.arange(N, dtype=np.int64)
    s_all = np.concatenate([np.asarray(src, np.int64), loops])
    d_all = np.concatenate([np.asarray(dst, np.int64), loops])

    deg = np.bincount(d_all, minlength=N)
    bins_dsts = _pack_bins(deg)

    perm = np.full(NCORES * DSTS, -1, dtype=np.int64)
    for b, dlist in enumerate(bins_dsts):
        core, blk = divmod(b, B)
        base = core * DSTS + blk * 128
        perm[base : base + len(dlist)] = dlist

    # CSR of edges by dst
    eorder = np.argsort(d_all, kind="stable")
    indptr = np.zeros(N + 1, dtype=np.int64)
    np.cumsum(deg, out=indptr[1:])
    s_sorted = s_all[eorder]

    node_bf = np.ascontiguousarray(x.astype(bf16_np))

    per_core = []
    for core in range(NCORES):
        srcmat = np.zeros((128, NPL), np.int64)
        dstmat = np.zeros((128, NPL), np.int64)
        dstloc = np.full((128, NPL), 992.0, dtype=np.float32)

        for blk in range(B):
            dlist = bins_dsts[core * B + blk]
            ss, dd_, dl = [], [], []
            for j, d0 in enumerate(dlist):
                es = s_sorted[indptr[d0] : indptr[d0 + 1]]
                ss.append(es)
                dd_.append(np.full(len(es), d0, np.int64))
                dl.append(np.full(len(es), j, np.float32))
            ss = np.concatenate(ss) if ss else np.zeros(0, np.int64)
            dd_ = np.concatenate(dd_) if dd_ else np.zeros(0, np.int64)
            dl = np.concatenate(dl) if dl else np.zeros(0, np.float32)
            ne = len(ss)
            assert ne <= TB * 128, f"block overflow {ne}"
            sp = np.zeros(TB * 128, np.int64)
            sp[:ne] = ss
            dp = np.zeros(TB * 128, np.int64)
            dp[:ne] = dd_
            lp = np.full(TB * 128, 992.0, np.float32)
            lp[:ne] = dl
            # edge slot j -> (plane j//128, partition j%128)
            pl0 = blk * TB
            srcmat[:, pl0 : pl0 + TB] = sp.reshape(TB, 128).T
            dstmat[:, pl0 : pl0 + TB] = dp.reshape(TB, 128).T
            dstloc[:, pl0 : pl0 + TB] = lp.reshape(TB, 128).T

        # edge-ordered feature streams
        g_src = node_bf[srcmat]                 # [128 e, NPL, F]
        g_dst = node_bf[dstmat]                 # [128 e, NPL, F]
        xem = np.ascontiguousarray(g_src.reshape(128, NPL * F))
        xtg = np.ascontiguousarray(
            g_src.transpose(2, 1, 0).reshape(128, NPL * 128)
        )  # [feat, (plane, edge)]
        xdt = np.ascontiguousarray(
            g_dst.transpose(2, 1, 0).reshape(128, NPL * 128)
        )

        nshT = np.zeros((F, DSTS), np.float32)
        sl = perm[core * DSTS : (core + 1) * DSTS]
        valid = sl >= 0
        nshT[:, valid] = x[sl[valid]].T

        per_core.append(
            dict(
                xem=xem,
                xtg=xtg,
                xdt=xdt,
                nshT=nshT.astype(bf16_np),
                dstloc=dstloc,
            )
        )

    return per_core, perm


def _build_nc():
    nc = bacc.Bacc(
        "TRN2",
        target_bir_lowering=False,
        debug=False,
        enable_asserts=False,
        num_devices=NCORES,
    )
    AF = mybir.ActivationFunctionType
    OP = mybir.AluOpType

    xem_d = nc.dram_tensor("xem", [128, NPL * F], bf16, kind="ExternalInput")
    xtg_d = nc.dram_tensor("xtg", [128, NPL * F], bf16, kind="ExternalInput")
    xdt_d = nc.dram_tensor("xdt", [128, NPL * F], bf16, kind="ExternalInput")
    nshT_d = nc.dram_tensor("nshT", [F, DSTS], bf16, kind="ExternalInput")
    dstloc_d = nc.dram_tensor("dstloc", [128, NPL], f32, kind="ExternalInput")
    wqT_d = nc.dram_tensor("wqT", [F, S], bf16, kind="ExternalInput")
    wkT_d = nc.dram_tensor("wkT", [F, S], bf16, kind="ExternalInput")
    w1T_d = nc.dram_tensor("w1T", [F, F], bf16, kind="ExternalInput")
    w2T_d = nc.dram_tensor("w2T", [F, F], bf16, kind="ExternalInput")
    bqk_d = nc.dram_tensor("bqk", [1, TB * 2 * S], bf16, kind="ExternalInput")
    b2r_d = nc.dram_tensor("b2r", [1, F], bf16, kind="ExternalInput")
    iota_d = nc.dram_tensor("iota", [128, 128], bf16, kind="ExternalInput")
    out_d = nc.dram_tensor("out", [DSTS, F], bf16, kind="ExternalOutput")

    with tile.TileContext(nc) as tc:
        with tc.tile_pool(name="const", bufs=1) as const:
            nshT = const.tile([F, DSTS], bf16)
            nc.sync.dma_start(nshT[:], nshT_d[:])
            dstloc = const.tile([128, NPL], f32)
            nc.sync.dma_start(dstloc[:], dstloc_d[:])
            wqT = const.tile([F, S], bf16)
            nc.sync.dma_start(wqT[:], wqT_d[:])
            wkT = const.tile([F, S], bf16)
            nc.sync.dma_start(wkT[:], wkT_d[:])
            w1T = const.tile([F, F], bf16)
            nc.sync.dma_start(w1T[:], w1T_d[:])
            w2T = const.tile([F, F], bf16)
            nc.sync.dma_start(w2T[:], w2T_d[:])
            bqk = const.tile([1, TB * 2 * S], bf16)
            nc.sync.dma_start(bqk[:], bqk_d[:])
            b2r = const.tile([1, F], bf16)
            nc.sync.dma_start(b2r[:], b2r_d[:])
            iota = const.tile([128, 128], bf16)
            nc.sync.dma_start(iota[:], iota_d[:])
            ones_col = const.tile([128, 1], bf16)
            nc.vector.memset(ones_col[:], 1.0)
            ones_row = const.tile([1, 128], bf16)
            nc.vector.memset(ones_row[:], 1.0)
            # per-core accumulators for the deferred l2-norm phase
            xpre_bf = const.tile([128, B * F], bf16)
            ssq_all = const.tile([128, B], f32)
            pb_all = const.tile([128, B * F], f32)
            ot_all = const.tile([128, B * F], bf16)

            with (
                tc.tile_pool(name="xe", bufs=2) as xep,
                tc.tile_pool(name="xt", bufs=2) as xtp,
                tc.tile_pool(name="xd", bufs=2) as xdp,
                tc.tile_pool(name="wk3", bufs=6) as wk3,
                tc.tile_pool(name="swp", bufs=8) as swp,
                tc.tile_pool(name="ps_qk", bufs=2, space="PSUM") as ps_qk,
                tc.tile_pool(name="ps_att", bufs=2, space="PSUM") as ps_att,
                tc.tile_pool(name="ps_ab", bufs=2, space="PSUM") as ps_ab,
            ):
                streams = {}

                def issue_group(g):
                    csl = slice(g * GPL * F, (g + 1) * GPL * F)
                    XEM = xep.tile([128, GPL * F], bf16, tag="XEM")
                    nc.sync.dma_start(XEM[:], xem_d[:, csl])
                    XTG = xtp.tile([128, GPL * F], bf16, tag="XTG")
                    nc.sync.dma_start(XTG[:], xtg_d[:, csl])
                    XDT = xdp.tile([128, GPL * F], bf16, tag="XDT")
                    nc.sync.dma_start(XDT[:], xdt_d[:, csl])
                    streams[g] = (XEM, XTG, XDT)

                def emit_qk(b):
                    """q/k matmuls + tanh/copy for block b (lookahead)."""
                    g, bb = divmod(b, G)
                    _, XTG, XDT = streams[g]
                    psqk = ps_qk.tile([128, TB, 2 * S], f32, tag="psqk")
                    nc.tensor.matmul(
                        psqk[:].rearrange("p t s -> p (t s)"),
                        ones_row[:], bqk[:],
                        start=True, stop=False, skip_group_check=True,
                    )
                    for t in range(TB):
                        fsl = slice((bb * TB + t) * F, (bb * TB + t + 1) * F)
                        nc.tensor.matmul(
                            psqk[:, t, 0:S], XTG[:, fsl], wqT[:],
                            start=False, stop=True, skip_group_check=True,
                        )
                        nc.tensor.matmul(
                            psqk[:, t, S : 2 * S], XDT[:, fsl], wkT[:],
                            start=False, stop=True, skip_group_check=True,
                        )
                    q_sb = wk3.tile([128, TB, S], bf16, tag="qsb")
                    nc.scalar.activation(q_sb[:], psqk[:, :, 0:S], AF.Tanh)
                    k_sb = wk3.tile([128, TB, S], bf16, tag="ksb")
                    nc.scalar.activation(
                        k_sb[:], psqk[:, :, S : 2 * S], AF.Copy
                    )
                    return q_sb, k_sb

                def emit_rest(b, qk):
                    g, bb = divmod(b, G)
                    XEM, _, _ = streams[g]
                    q_sb, k_sb = qk
                    prod = wk3.tile([128, TB, S], bf16, tag="prod")
                    nc.vector.tensor_tensor(prod[:], q_sb[:], k_sb[:], OP.mult)
                    scores = wk3.tile([128, TB], f32, tag="scores")
                    nc.vector.tensor_reduce(
                        scores[:], prod[:], mybir.AxisListType.X, OP.add
                    )
                    expb = wk3.tile([128, TB], f32, tag="expb")
                    nc.scalar.activation(
                        expb[:], scores[:], AF.Exp, scale=INV_SQRT_S
                    )

                    att = ps_att.tile([F, 128], f32, tag="att")
                    paseg = ps_ab.tile([128, F + 1], f32, tag="paseg")
                    seg = paseg[:, F : F + 1]
                    pa = paseg[:, 0:F]
                    for t in range(TB):
                        fsl = slice((bb * TB + t) * F, (bb * TB + t + 1) * F)
                        sw = swp.tile([128, 128], bf16, tag="sw")
                        nc.vector.tensor_scalar(
                            sw[:],
                            iota[:],
                            dstloc[:, b * TB + t : b * TB + t + 1],
                            expb[:, t : t + 1],
                            OP.is_equal,
                            OP.mult,
                        )
                        nc.tensor.matmul(
                            att[:], XEM[:, fsl], sw[:],
                            start=(t == 0), stop=(t == TB - 1),
                        )
                        nc.tensor.matmul(
                            seg, sw[:], ones_col[:],
                            start=(t == 0), stop=(t == TB - 1),
                            skip_group_check=True,
                        )

                    attT_sb = wk3.tile([F, 128], bf16, tag="attTsb")
                    nc.scalar.activation(attT_sb[:], att[:], AF.Copy)
                    nc.tensor.matmul(
                        pa, attT_sb[:], w2T[:], start=True, stop=True,
                        skip_group_check=True,
                    )
                    rec = wk3.tile([128, 1], f32, tag="rec")
                    nc.vector.reciprocal(rec[:], seg)
                    # xpre = pa/seg + pb, stored bf16 for the final phase
                    nc.vector.scalar_tensor_tensor(
                        xpre_bf[:, b * F : (b + 1) * F],
                        pa, rec[:], pb_all[:, b * F : (b + 1) * F],
                        OP.mult, OP.add,
                    )
                    sq_scr = wk3.tile([128, F], bf16, tag="sqscr")
                    nc.scalar.activation(
                        sq_scr[:],
                        xpre_bf[:, b * F : (b + 1) * F],
                        AF.Square,
                        accum_out=ssq_all[:, b : b + 1],
                    )

                out_v = out_d[:].rearrange("(b p) f -> p b f", p=128)
                ot_v = ot_all[:].rearrange("p (b f) -> p b f", f=F)

                def emit_norm(b0, b1):
                    """Normalize + relu + out-DMA for blocks [b0, b1)."""
                    nrm = wk3.tile([128, B], f32, tag="nrm")
                    nc.scalar.activation(
                        nrm[:, b0:b1], ssq_all[:, b0:b1], AF.Sqrt
                    )
                    rin = wk3.tile([128, B], f32, tag="rin")
                    nc.vector.reciprocal(rin[:, b0:b1], nrm[:, b0:b1])
                    for b in range(b0, b1):
                        nc.vector.tensor_scalar(
                            ot_all[:, b * F : (b + 1) * F],
                            xpre_bf[:, b * F : (b + 1) * F],
                            rin[:, b : b + 1],
                            0.0,
                            OP.mult,
                            OP.max,
                        )
                    CB = 10
                    for c in range(b0 // CB, b1 // CB):
                        nc.sync.dma_start(
                            out_v[:, c * CB : (c + 1) * CB, :],
                            ot_v[:, c * CB : (c + 1) * CB, :],
                        )

                issue_group(0)
                # upfront pb = x@W1T + b2 (PE busy while streams land)
                with tc.tile_pool(name="ps_pb", bufs=2, space="PSUM") as ps_pb:
                    for b0 in range(B):
                        pbp = ps_pb.tile([128, F], f32, tag="pbp")
                        nc.tensor.matmul(
                            pbp[:],
                            nshT[:, b0 * 128 : (b0 + 1) * 128],
                            w1T[:],
                            start=True,
                            stop=False,
                        )
                        nc.tensor.matmul(
                            pbp[:], ones_row[:, :F], b2r[:],
                            start=False, stop=True,
                        )
                        nc.scalar.activation(
                            pb_all[:, b0 * F : (b0 + 1) * F], pbp[:],
                            AF.Copy,
                        )

                # software-pipelined emission: q/k of block b+1 queued on
                # PE before block b's sw-gated att/seg, so neither engine
                # ping-pongs on the other once per block.
                qk = emit_qk(0)
                for b in range(B):
                    g = b // G
                    if b % G == 0 and g + 1 < NG:
                        issue_group(g + 1)
                    qk_next = emit_qk(b + 1) if b + 1 < B else None
                    emit_rest(b, qk)
                    qk = qk_next
                emit_norm(0, B)


    nc.compile()
    return nc


def get_nc():
    if "nc" not in _compiled:
        _compiled["nc"] = _build_nc()
    return _compiled["nc"]


def _make_in_maps(node_data, src, dst, Wq, bq, Wk, bk, W1, W2, b2):
    per_core, perm = _host_prep(node_data, src, dst)
    consts = dict(
        wqT=np.ascontiguousarray(np.asarray(Wq, np.float32).T).astype(bf16_np),
        wkT=np.ascontiguousarray(np.asarray(Wk, np.float32).T).astype(bf16_np),
        w1T=np.ascontiguousarray(np.asarray(W1, np.float32).T).astype(bf16_np),
        w2T=np.ascontiguousarray(np.asarray(W2, np.float32).T).astype(bf16_np),
        bqk=np.tile(
            np.concatenate([np.asarray(bq, np.float32),
                            np.asarray(bk, np.float32)])[None, :],
            (1, TB),
        ).astype(bf16_np),
        b2r=np.ascontiguousarray(
            np.asarray(b2, np.float32)[None, :]
        ).astype(bf16_np),
        iota=np.tile(
            np.arange(128, dtype=np.float32)[None, :], (128, 1)
        ).astype(bf16_np),
    )
    in_maps = []
    for core in range(NCORES):
        m = dict(consts)
        m.update(per_core[core])
        in_maps.append(m)
    return in_maps, perm


def run(node_data, src, dst, Wq, bq, Wk, bk, W1, W2, b2, trace=False,
        tmpdir=None, n_runs=1):
    in_maps, perm = _make_in_maps(
        node_data, src, dst, Wq, bq, Wk, bk, W1, W2, b2
    )
    nc = get_nc()
    res = None
    for r in range(n_runs):
        td = tmpdir if (tmpdir is None or n_runs == 1) else f"{tmpdir}_{r}"
        if td is not None:
            import os
            import shutil
            shutil.rmtree(td, ignore_errors=True)
            os.makedirs(td, exist_ok=True)
        rr = run_bass_kernel_spmd(
            nc, in_maps, list(range(NCORES)), trace=trace, tmpdir=td
        )
        if rr.exec_time_ns is not None:
            print(f"  run {r}: {rr.exec_time_ns} ns")
        if res is None or (
            rr.exec_time_ns is not None
            and res.exec_time_ns is not None
            and rr.exec_time_ns < res.exec_time_ns
        ):
            res = rr
    out = np.zeros((N, F), dtype=np.float32)
    for core in range(NCORES):
        o = np.asarray(res.results[core]["out"], dtype=np.float32)
        sl = perm[core * DSTS : (core + 1) * DSTS]
        valid = sl >= 0
        out[sl[valid]] = o[valid]
    return out, res


def kernel(node_data, src, dst, Wq, bq, Wk, bk, W1, W2, b2):
    out, _ = run(node_data, src, dst, Wq, bq, Wk, bk, W1, W2, b2, trace=False)
    return out


if __name__ == "__main__":
    nc = get_nc()
    print("compiled OK")
